# revision 6
# baseline (speedup 1.0000x reference)
"""Trainium2 Bass kernel for nn_CNNEncoder (gather -> lin1 -> conv1d -> maxpool -> MLP).

Strategy (v3)
-------------
Data-parallel over the 1024 = 64*16 sentences: 128 sentences per NeuronCore.

Host-side algebra: lin1 is folded into the conv weights (see below); the
constant bias commutes with max-over-time and is folded into the MLP bias.

Key device-side idea: the embedding gather uses dma_gather(transpose=True),
which gathers rows from a per-core COMPACT table (unique tokens of this
core's 128 sentences, <= 16384 rows so int16-indexable) and transposes them
to channel-major during the DMA:

    eg[p, j, s*L + t] = table[idx[s*L+t], j*128 + p]   (bf16)

so the conv can consume [channel, (sent, pos)] tiles directly -- no PE
transposes and no per-sentence SWDGE gathers (994ns fixed overhead each).

Conv: y[o,t] = sum_{k,i} e[i, t+k] * Weff[k][i, o], contraction 300ch x 5taps
= 1500 rows -> 12 PSUM-accumulated bf16 matmuls per (o_chunk, 4-sentence
block): 10 full 128-row (chunk j, tap k) matmuls slice the gathered tile at
column offset k (shift = free AP offset), and the 44-channel tail (220 rows)
is packed into 128+92-row matmuls whose tap shifts are baked by SBUF->SBUF
DMA column-shift copies (T1/T2 tiles). 12 matmuls x 3 o-passes x 496 cols is
the PE floor for this contraction (ceil(1500/128) x ceil(300/128)).

Tail: DVE max over time -> concat with mention -> tanh MLP in f32r (as v1).
"""

import sys

sys.path.insert(0, "/opt/trn_rl_repo")

from contextlib import ExitStack

import numpy as np
import ml_dtypes

import concourse.bass as bass
import concourse.mybir as mybir
import concourse.tile as tile
from concourse import bacc, bass_utils

F32 = mybir.dt.float32
F32R = mybir.dt.float32r
BF16 = mybir.dt.bfloat16
I16 = mybir.dt.int16

VOCAB = 100000
D = 300
K = 5
L = 128          # tokens per sentence
NSENT = 1024     # total sentences
NCORES = 8
NS = NSENT // NCORES   # sentences per core = 128
SB = 4                 # sentences per conv block
TP = L - K + 1         # 124 valid conv positions
CH = [(0, 128), (128, 256), (256, 300)]  # output-channel passes

UPAD = 16384           # compact table rows (uniform across cores)
E = 384                # table row elements (bf16) = 768B, 256B-multiple
GS = 4                 # sentences per gather group (>512 idxs/gather wedges HW)
NG = NS // GS          # 32 gather groups
NBG = GS // SB         # 1 conv block per group

_PROGRAM_CACHE = {}


def _build_program() -> bass.Bass:
    nc = bacc.Bacc(None, target_bir_lowering=False, dynamic_dma_scratch_size=65536)

    # ---- per-core DRAM I/O ----
    tabc = nc.dram_tensor("tabc", [UPAD, E], BF16, kind="ExternalInput")
    idx_d = nc.dram_tensor("idx", [128, NG * (GS * L // 16)], I16, kind="ExternalInput")
    # main conv weights: [j*5+k, 128 i-rows, 300 o]; tail packed tiles
    wm_d = nc.dram_tensor("wm", [128, 10, D], BF16, kind="ExternalInput")
    t1w_d = nc.dram_tensor("t1w", [128, D], BF16, kind="ExternalInput")
    t2w_d = nc.dram_tensor("t2w", [92, D], BF16, kind="ExternalInput")
    idn = nc.dram_tensor("idn", [L, L], F32R, kind="ExternalInput")
    # tail weights with biases folded as an extra contraction row
    w2cat = nc.dram_tensor("w2cat", [2 * D + 1, D], F32R, kind="ExternalInput")
    w3cat = nc.dram_tensor("w3cat", [D + 1, D], F32R, kind="ExternalInput")
    m_t = nc.dram_tensor("mt", [D + 1, NS], F32R, kind="ExternalInput")
    out_d = nc.dram_tensor("out", [NS, D], F32, kind="ExternalOutput")

    GCOLS = GS * L // 16  # idx cols per group

    with tile.TileContext(nc) as tc, ExitStack() as ctx:
        const = ctx.enter_context(tc.tile_pool(name="const", bufs=1))
        epool = ctx.enter_context(tc.tile_pool(name="e", bufs=2))
        tpool = ctx.enter_context(tc.tile_pool(name="t", bufs=2))
        pspool = ctx.enter_context(tc.tile_pool(name="ps", bufs=6, space="PSUM"))

        idx_sb = const.tile([128, NG * GCOLS], I16)
        nc.sync.dma_start(out=idx_sb[:], in_=idx_d[:])

        wm_sb = const.tile([128, 10, D], BF16)
        nc.sync.dma_start(out=wm_sb[:], in_=wm_d[:])
        t1w_sb = const.tile([128, D], BF16)
        nc.sync.dma_start(out=t1w_sb[:], in_=t1w_d[:])
        t2w_sb = const.tile([92, D], BF16)
        nc.sync.dma_start(out=t2w_sb[:], in_=t2w_d[:])

        ident = const.tile([128, 128], F32R)
        nc.sync.dma_start(out=ident[:], in_=idn[:])

        # concat_T tiles [row-chunk, sent] for the 601-row tail contraction:
        # rows 0:300 cnn (written by conv reduce_max), 300:600 mention, 600 ones
        W2CH = [(0, 128), (128, 256), (256, 384), (384, 512), (512, 601)]
        c_sb = [
            const.tile([c1 - c0, NS], F32R, tag=f"c_{c0}", name=f"c_{c0}")
            for c0, c1 in W2CH
        ]
        nc.sync.dma_start(out=c_sb[2][44:128, :], in_=m_t[0:84, :])
        nc.sync.dma_start(out=c_sb[3][:], in_=m_t[84:212, :])
        nc.sync.dma_start(out=c_sb[4][:], in_=m_t[212:301, :])

        w2cat_sb = []
        for c0, c1 in W2CH:
            t = const.tile([c1 - c0, D], F32R, tag=f"w2c_{c0}", name=f"w2c_{c0}")
            nc.sync.dma_start(out=t[:], in_=w2cat[c0:c1, :])
            w2cat_sb.append(t)

        JCH = [(0, 100), (100, 200), (200, 300)]
        w3cat_sb = []
        for j0, j1 in JCH:
            t = const.tile([j1 - j0, D], F32R, tag=f"w3c_{j0}", name=f"w3c_{j0}")
            nc.sync.dma_start(out=t[:], in_=w3cat[j0:j1, :])
            w3cat_sb.append(t)
        b3row_sb = const.tile([1, D], F32R)
        nc.sync.dma_start(out=b3row_sb[:], in_=w3cat[D : D + 1, :])
        ones_sb = const.tile([1, NS], F32R)
        nc.sync.dma_start(out=ones_sb[:], in_=m_t[D : D + 1, :])

        # ---- main loop over gather groups of GS sentences ----
        for g in range(NG):
            eg = epool.tile([128, 3, GS, L], BF16, tag="eg", name=f"eg{g}")
            nc.gpsimd.dma_gather(
                out_ap=eg[:].rearrange("p j s t -> p j (s t)"),
                in_ap=tabc[:],
                idxs_ap=idx_sb[:, g * GCOLS : (g + 1) * GCOLS],
                num_idxs=GS * L,
                num_idxs_reg=GS * L,
                elem_size=E,
                transpose=True,
            )

            # tail tiles: 44 channels (256:300) x 5 taps packed into 128+92
            # rows, tap shift delta baked as a column offset during the copy.
            t1 = tpool.tile([128, GS, L], BF16, tag="t1", name=f"t1_{g}")
            t2 = tpool.tile([128, GS, L], BF16, tag="t2", name=f"t2_{g}")
            # (dest tile, dest row0, src ch0, src ch1, tap shift)
            for dst, r0, c0, c1, dlt in (
                (t1, 0, 0, 44, 0),
                (t1, 44, 0, 44, 1),
                (t1, 88, 0, 40, 2),
                (t2, 0, 40, 44, 2),
                (t2, 4, 0, 44, 3),
                (t2, 48, 0, 44, 4),
            ):
                n = c1 - c0
                nc.sync.dma_start(
                    out=dst[r0 : r0 + n, :, 0 : L - dlt],
                    in_=eg[c0:c1, 2, :, dlt:L],
                )

            for b in range(NBG):
                s0 = b * SB
                for oi, (o0, o1) in enumerate(CH):
                    ps_y = pspool.tile([o1 - o0, SB, TP], F32, tag="ps")
                    n = 0
                    for j in range(2):
                        for k in range(K):
                            nc.tensor.matmul(
                                out=ps_y[:],
                                lhsT=wm_sb[:, j * 5 + k, o0:o1],
                                rhs=eg[:, j, s0 : s0 + SB, k : k + TP],
                                start=(n == 0),
                                stop=False,
                            )
                            n += 1
                    nc.tensor.matmul(
                        out=ps_y[:],
                        lhsT=t1w_sb[:, o0:o1],
                        rhs=t1[:, s0 : s0 + SB, 0:TP],
                        start=False,
                        stop=False,
                    )
                    nc.tensor.matmul(
                        out=ps_y[:],
                        lhsT=t2w_sb[:, o0:o1],
                        rhs=t2[0:92, s0 : s0 + SB, 0:TP],
                        start=False,
                        stop=True,
                    )
                    cnn_rows = c_sb[oi][0 : o1 - o0] if oi == 2 else c_sb[oi][:]
                    col = g * GS + s0
                    nc.vector.tensor_reduce(
                        out=cnn_rows[:, col : col + SB],
                        in_=ps_y[:],
                        axis=mybir.AxisListType.X,
                        op=mybir.AluOpType.max,
                    )

        # ---- tail MLP, f32r full-rate (N=300), biases folded as ones-rows ----
        ps_h = pspool.tile([NS, D], F32, tag="ps")
        for c, (c0, c1) in enumerate(W2CH):
            nc.tensor.matmul(
                out=ps_h[:],
                lhsT=c_sb[c][:],
                rhs=w2cat_sb[c][:],
                start=(c == 0),
                stop=(c == len(W2CH) - 1),
            )
        h_sb = const.tile([NS, D], F32R)
        nc.scalar.activation(
            out=h_sb[:], in_=ps_h[:], func=mybir.ActivationFunctionType.Tanh
        )

        ht_sb = []
        for jc, (j0, j1) in enumerate(JCH):
            ps_ht = pspool.tile([100, NS], F32R, tag="ps")
            nc.tensor.transpose(out=ps_ht[:], in_=h_sb[:, j0:j1], identity=ident[:])
            ht = const.tile([100, NS], F32R, tag=f"ht_{j0}", name=f"ht_{j0}")
            nc.scalar.copy(out=ht[:], in_=ps_ht[:])
            ht_sb.append(ht)

        ps_o = pspool.tile([NS, D], F32, tag="ps")
        for jc in range(3):
            nc.tensor.matmul(
                out=ps_o[:],
                lhsT=ht_sb[jc][:],
                rhs=w3cat_sb[jc][:],
                start=(jc == 0),
                stop=False,
            )
        nc.tensor.matmul(
            out=ps_o[:], lhsT=ones_sb[:], rhs=b3row_sb[:], start=False, stop=True
        )
        out_sb = const.tile([NS, D], F32)
        nc.scalar.copy(out=out_sb[:], in_=ps_o[:])
        nc.sync.dma_start(out=out_d[:], in_=out_sb[:])

    nc.finalize()
    return nc


def get_program() -> bass.Bass:
    if "v3" not in _PROGRAM_CACHE:
        _PROGRAM_CACHE["v3"] = _build_program()
    return _PROGRAM_CACHE["v3"]


def _prepare_in_maps(inputs: dict) -> list[dict]:
    token_ids = np.asarray(inputs["token_ids"]).astype(np.int64)      # [1024, 128]
    mention = np.asarray(inputs["mention_rep"], dtype=np.float32).reshape(NSENT, D)
    emb = np.asarray(inputs["emb"], dtype=np.float32)
    W1 = np.asarray(inputs["W1"], dtype=np.float64)
    b1 = np.asarray(inputs["b1"], dtype=np.float64)
    conv_w = np.asarray(inputs["conv_w"], dtype=np.float64)           # [o, i, k]
    conv_b = np.asarray(inputs["conv_b"], dtype=np.float64)
    W2 = np.asarray(inputs["W2"], dtype=np.float64)                   # [2D, D]
    b2 = np.asarray(inputs["b2"], dtype=np.float64)
    W3 = np.asarray(inputs["W3"], dtype=np.float64)                   # [j, q]
    b3 = np.asarray(inputs["b3"], dtype=np.float64)

    Wk = conv_w.transpose(1, 0, 2)                                    # [i, o, k]
    weff = np.stack([W1 @ Wk[:, :, k] for k in range(K)])             # [k, i, o]
    beff = b1 @ Wk.sum(axis=2) + conv_b                               # [o]
    b2eff = b2 + beff @ W2[:D]                                        # [j]
    w2cat_h = np.concatenate([W2, b2eff[None, :]], axis=0).astype(np.float32)
    w3cat_h = np.concatenate([W3, b3[None, :]], axis=0).astype(np.float32)

    weff_bf = weff.astype(ml_dtypes.bfloat16)
    wm_h = np.zeros((128, 10, D), ml_dtypes.bfloat16)
    for j in range(2):
        for k in range(K):
            wm_h[:, j * 5 + k, :] = weff_bf[k, j * 128 : (j + 1) * 128, :]
    t1w_h = np.zeros((128, D), ml_dtypes.bfloat16)
    t1w_h[0:44] = weff_bf[0, 256:300]
    t1w_h[44:88] = weff_bf[1, 256:300]
    t1w_h[88:128] = weff_bf[2, 256:296]
    t2w_h = np.zeros((92, D), ml_dtypes.bfloat16)
    t2w_h[0:4] = weff_bf[2, 296:300]
    t2w_h[4:48] = weff_bf[3, 256:300]
    t2w_h[48:92] = weff_bf[4, 256:300]

    idn_h = np.eye(L, dtype=np.float32)

    in_maps = []
    for c in range(NCORES):
        sl = slice(c * NS, (c + 1) * NS)
        tids = token_ids[sl]                                          # [128, 128]
        uniq, inv = np.unique(tids.ravel(), return_inverse=True)
        assert uniq.size <= UPAD
        tab_h = np.zeros((UPAD, E), ml_dtypes.bfloat16)
        tab_h[: uniq.size, :D] = emb[uniq].astype(ml_dtypes.bfloat16)
        # idx wrap: flat i (= local_sent*L + pos) -> col i//16, row i%16,
        # replicated across the 8 partition groups of 16
        cid = inv.reshape(NS, L).astype(np.int16)                     # [sent, pos]
        idx_h = np.zeros((128, NG * GCOLS_H), np.int16)
        for g in range(NG):
            cg = cid[g * GS : (g + 1) * GS].ravel()                   # [GS*L]
            wrap = cg.reshape(GCOLS_H, 16).T                          # [16, GCOLS]
            idx_h[:, g * GCOLS_H : (g + 1) * GCOLS_H] = np.tile(wrap, (8, 1))

        mt_h = np.ones((D + 1, NS), np.float32)
        mt_h[:D] = mention[sl].T
        in_maps.append(
            {
                "tabc": tab_h,
                "idx": idx_h,
                "wm": wm_h,
                "t1w": t1w_h,
                "t2w": t2w_h,
                "idn": idn_h,
                "w2cat": w2cat_h,
                "w3cat": w3cat_h,
                "mt": mt_h,
            }
        )
    return in_maps


GCOLS_H = GS * L // 16


def run(inputs: dict, trace: bool = False, **kwargs):
    """Run the kernel; returns (output [1024, 300] f32, BassKernelResults)."""
    nc = get_program()
    in_maps = _prepare_in_maps(inputs)
    res = bass_utils.run_bass_kernel_spmd(
        nc, in_maps, core_ids=list(range(NCORES)), trace=trace, **kwargs
    )
    out = np.concatenate(
        [np.asarray(r["out"]) for r in res.results], axis=0
    ).astype(np.float32)
    return out, res


def kernel(**inputs) -> np.ndarray:
    out, _ = run(inputs)
    return out


# revision 24
# speedup vs baseline: 1.1596x; 1.1596x over previous
"""Trainium2 Bass kernel for nn_CNNEncoder (gather -> lin1 -> conv1d -> maxpool -> MLP).

Strategy (v4)
-------------
Data-parallel over the 1024 = 64*16 sentences: 128 sentences per NeuronCore.

Host-side algebra: lin1 is folded into the conv weights; the constant conv
bias commutes with max-over-time and is folded into the MLP bias.

Embedding gather: dma_gather(transpose=True) from a per-core COMPACT table
(unique tokens of this core's sentences, <= 16384 rows so int16-indexable),
which lands CHANNEL-MAJOR bf16 tiles directly:

    eg[p, j, s*L + t] = table[idx[s*L+t], j*128 + p]

No PE transposes, no per-row SWDGE gathers (994ns fixed overhead each); one
512-token gather per 4-sentence block (>512 idxs per gather wedges HW).

Conv: contraction 300ch x 5taps = 1500 rows at the PE structural floor of
12 PSUM-accumulated bf16 matmuls per (o_pass, block): 10 full 128-row
(chunk, tap) matmuls slice eg at column offset k, and the 44-channel tail
(220 rows) packs into 128+92-row matmuls whose tap shifts are baked by
SBUF->SBUF DMA column-shift copies (T1/T2). 12mm x 3 passes x 496 cols.

Tail MLP entirely in bf16 and TRANSPOSED (h_T = tanh(W2^T @ concat_T)) so no
PE transposes / PSUM round-trips; mention-only contraction chunks run early
(hidden under the conv); biases ride as ones-rows.
"""

import sys

sys.path.insert(0, "/opt/trn_rl_repo")

from contextlib import ExitStack

import numpy as np
import ml_dtypes

import concourse.bass as bass
import concourse.mybir as mybir
import concourse.tile as tile
from concourse import bacc, bass_utils

F32 = mybir.dt.float32
F32R = mybir.dt.float32r
BF16 = mybir.dt.bfloat16
I16 = mybir.dt.int16

VOCAB = 100000
D = 300
K = 5
L = 128          # tokens per sentence
NSENT = 1024     # total sentences
NCORES = 8
NS = NSENT // NCORES   # sentences per core = 128
SB = 4                 # sentences per conv block
TP = L - K + 1         # 124 valid conv positions
CH = [(0, 128), (128, 256), (256, 300)]   # conv output-channel passes
JCH = [(0, 128), (128, 256), (256, 300)]  # MLP hidden chunks

UPAD = 16384           # compact table rows (uniform across cores)
E = 384                # table row elements (bf16) = 768B, 256B-multiple
GS = 4                 # sentences per gather (>512 idxs/gather wedges HW)
NG = NS // GS          # 32 gather groups (= conv blocks)
GCOLS = GS * L // 16   # idx columns per group (32)

_PROGRAM_CACHE = {}


def _build_program() -> bass.Bass:
    nc = bacc.Bacc(None, target_bir_lowering=False, dynamic_dma_scratch_size=65536)

    tabc = nc.dram_tensor("tabc", [UPAD, E], BF16, kind="ExternalInput")
    idx0_d = nc.dram_tensor("idx0", [128, GCOLS], I16, kind="ExternalInput")
    idxr_d = nc.dram_tensor("idxr", [128, (NG - 1) * GCOLS], I16, kind="ExternalInput")
    wm_d = nc.dram_tensor("wm", [128, 10, D], BF16, kind="ExternalInput")
    t1w_d = nc.dram_tensor("t1w", [128, D], BF16, kind="ExternalInput")
    t2w_d = nc.dram_tensor("t2w", [92, D], BF16, kind="ExternalInput")
    # W2 with b2eff folded: rows 0:512 as [128, 4, 300], rows 512:601 flat
    wa_d = nc.dram_tensor("wa", [128, 4, D], F32R, kind="ExternalInput")
    wb_d = nc.dram_tensor("wb", [89, D], F32R, kind="ExternalInput")
    # W3 rows 0:256 as [128, 2, 300]; rows 256:300 + b3 row -> [45, 300]
    w3ab_d = nc.dram_tensor("w3ab", [128, 2, D], F32R, kind="ExternalInput")
    w3c_d = nc.dram_tensor("w3c", [45, D], F32R, kind="ExternalInput")
    m_t = nc.dram_tensor("mt", [D + 1, NS], F32R, kind="ExternalInput")
    idn = nc.dram_tensor("idn", [L, L], F32R, kind="ExternalInput")
    out_d = nc.dram_tensor("out", [NS, D], F32, kind="ExternalOutput")

    with tile.TileContext(nc) as tc, ExitStack() as ctx:
        const = ctx.enter_context(tc.tile_pool(name="const", bufs=1))
        epool = ctx.enter_context(tc.tile_pool(name="e", bufs=3))
        tpool = ctx.enter_context(tc.tile_pool(name="t", bufs=3))
        ypool = ctx.enter_context(tc.tile_pool(name="y", bufs=2))
        pspool = ctx.enter_context(tc.tile_pool(name="ps", bufs=5, space="PSUM"))
        zpool = ctx.enter_context(tc.tile_pool(name="z", bufs=1, space="PSUM"))

        # -- startup-critical loads first: idx of group 0, conv weights --
        idx0_sb = const.tile([128, GCOLS], I16)
        nc.sync.dma_start(out=idx0_sb[:], in_=idx0_d[:])
        wm_sb = const.tile([128, 10, D], BF16)
        nc.sync.dma_start(out=wm_sb[:], in_=wm_d[:])
        t1w_sb = const.tile([128, D], BF16)
        nc.sync.dma_start(out=t1w_sb[:], in_=t1w_d[:])
        t2w_sb = const.tile([92, D], BF16)
        nc.sync.dma_start(out=t2w_sb[:], in_=t2w_d[:])
        ident = const.tile([L, L], F32R)
        nc.sync.dma_start(out=ident[:], in_=idn[:])

        # concat_T row-chunks [c, sent] bf16: rows 0:300 cnn (conv writes),
        # 300:600 mention, 600 ones
        W2CH = [(0, 128), (128, 256), (256, 384), (384, 512), (512, 601)]
        c_sb = [
            const.tile([c1 - c0, NS], F32R, tag=f"c_{c0}", name=f"c_{c0}")
            for c0, c1 in W2CH
        ]

        idxr_sb = const.tile([128, (NG - 1) * GCOLS], I16)
        wa_sb = const.tile([128, 4, D], F32R)
        wb_sb = const.tile([89, D], F32R)
        w3ab_sb = const.tile([128, 2, D], F32R)
        w3c_sb = const.tile([45, D], F32R)
        # h_T chunks; h2t row 44 is the ones row driving the b3 contraction
        h_sb = [
            const.tile([128, NS], F32R, tag="h0", name="h0"),
            const.tile([128, NS], F32R, tag="h1", name="h1"),
            const.tile([45, NS], F32R, tag="h2", name="h2"),
        ]

        z_ps = [
            zpool.tile([j1 - j0, NS], F32, tag=f"z{jc}", name=f"z{jc}")
            for jc, (j0, j1) in enumerate(JCH)
        ]

        def gather_group(g):
            eg = epool.tile([128, 3, GS, L], BF16, tag="eg", name=f"eg{g}")
            idxs = (
                idx0_sb[:]
                if g == 0
                else idxr_sb[:, (g - 1) * GCOLS : g * GCOLS]
            )
            nc.gpsimd.dma_gather(
                out_ap=eg[:].rearrange("p j s t -> p j (s t)"),
                in_ap=tabc[:],
                idxs_ap=idxs,
                num_idxs=GS * L,
                num_idxs_reg=GS * L,
                elem_size=E,
                transpose=True,
            )
            t1 = tpool.tile([128, GS, L], BF16, tag="t1", name=f"t1_{g}")
            t2 = tpool.tile([128, GS, L], BF16, tag="t2", name=f"t2_{g}")
            for dst, r0, c0, c1, dlt in (
                (t1, 0, 0, 44, 0),
                (t1, 44, 0, 44, 1),
                (t1, 88, 0, 40, 2),
                (t2, 0, 40, 44, 2),
                (t2, 4, 0, 44, 3),
                (t2, 48, 0, 44, 4),
            ):
                n = c1 - c0
                nc.sync.dma_start(
                    out=dst[r0 : r0 + n, :, 0 : L - dlt],
                    in_=eg[c0:c1, 2, :, dlt:L],
                )
            return eg, t1, t2

        import os

        flip3 = os.environ.get("KFLIP3", "1") == "1"

        def conv_group(g, eg, t1, t2):
            col = g * GS
            if not flip3:
                for oi, (o0, o1) in enumerate(CH):
                    ps_y = pspool.tile([o1 - o0, SB, TP], F32, tag="ps")
                    n = 0
                    for j in range(2):
                        for k in range(K):
                            nc.tensor.matmul(
                                out=ps_y[:],
                                lhsT=wm_sb[:, j * 5 + k, o0:o1],
                                rhs=eg[:, j, :, k : k + TP],
                                start=(n == 0), stop=False,
                            )
                            n += 1
                    nc.tensor.matmul(
                        out=ps_y[:], lhsT=t1w_sb[:, o0:o1], rhs=t1[:, :, 0:TP],
                        start=False, stop=False,
                    )
                    nc.tensor.matmul(
                        out=ps_y[:], lhsT=t2w_sb[:, o0:o1], rhs=t2[0:92, :, 0:TP],
                        start=False, stop=True,
                    )
                    rows = c_sb[oi][0 : o1 - o0] if oi == 2 else c_sb[oi][:]
                    nc.vector.tensor_reduce(
                        out=rows[:, col : col + SB], in_=ps_y[:],
                        axis=mybir.AxisListType.X, op=mybir.AluOpType.max,
                    )
                return
            # pass 3 (44 out channels) FLIPPED: out = [t, o] per sentence so
            # the moving dim is 44 (mm cost ∝ moving size only); transposed
            # back below for the free-axis max. Emitted first so the ACT
            # copies and transposes hide under passes 1-2.
            y3sb = ypool.tile([TP, SB, 44], F32R, tag="y3", name=f"y3_{g}")
            for s in range(SB):
                ps3 = pspool.tile([TP, 44], F32, tag="ps")
                n = 0
                for j in range(2):
                    for k in range(K):
                        nc.tensor.matmul(
                            out=ps3[:],
                            lhsT=eg[:, j, s, k : k + TP],
                            rhs=wm_sb[:, j * 5 + k, 256:300],
                            start=(n == 0),
                            stop=False,
                        )
                        n += 1
                nc.tensor.matmul(
                    out=ps3[:], lhsT=t1[:, s, 0:TP], rhs=t1w_sb[:, 256:300],
                    start=False, stop=False,
                )
                nc.tensor.matmul(
                    out=ps3[:], lhsT=t2[0:92, s, 0:TP], rhs=t2w_sb[:, 256:300],
                    start=False, stop=True,
                )
                nc.scalar.copy(out=y3sb[:, s, :], in_=ps3[:])
            # passes 1-2 (out channels 0:128, 128:256): out = [o, sent, t]
            for oi, (o0, o1) in enumerate(CH[:2]):
                ps_y = pspool.tile([o1 - o0, SB, TP], F32, tag="ps")
                n = 0
                for j in range(2):
                    for k in range(K):
                        nc.tensor.matmul(
                            out=ps_y[:],
                            lhsT=wm_sb[:, j * 5 + k, o0:o1],
                            rhs=eg[:, j, :, k : k + TP],
                            start=(n == 0),
                            stop=False,
                        )
                        n += 1
                nc.tensor.matmul(
                    out=ps_y[:], lhsT=t1w_sb[:, o0:o1], rhs=t1[:, :, 0:TP],
                    start=False, stop=False,
                )
                nc.tensor.matmul(
                    out=ps_y[:], lhsT=t2w_sb[:, o0:o1], rhs=t2[0:92, :, 0:TP],
                    start=False, stop=True,
                )
                nc.vector.tensor_reduce(
                    out=c_sb[oi][:, col : col + SB],
                    in_=ps_y[:],
                    axis=mybir.AxisListType.X,
                    op=mybir.AluOpType.max,
                )
            ps3t = pspool.tile([44, SB, TP], F32R, tag="ps")
            for s in range(SB):
                nc.tensor.transpose(
                    out=ps3t[:, s, :], in_=y3sb[:, s, :], identity=ident[0:TP, 0:TP]
                )
            nc.vector.tensor_reduce(
                out=c_sb[2][0:44, col : col + SB],
                in_=ps3t[:],
                axis=mybir.AxisListType.X,
                op=mybir.AluOpType.max,
            )

        # groups 0..1: prime the pipeline before emitting the late consts
        pending = [gather_group(0)]
        nc.sync.dma_start(out=idxr_sb[:], in_=idxr_d[:])
        pending.append(gather_group(1))
        conv_group(0, *pending[0])

        # -- late consts: needed only by the MLP tail / later groups --
        nc.sync.dma_start(out=c_sb[2][44:128, :], in_=m_t[0:84, :])
        nc.sync.dma_start(out=c_sb[3][:], in_=m_t[84:212, :])
        nc.sync.dma_start(out=c_sb[4][:], in_=m_t[212:301, :])
        nc.sync.dma_start(out=wa_sb[:], in_=wa_d[:])
        nc.sync.dma_start(out=wb_sb[:], in_=wb_d[:])
        nc.sync.dma_start(out=w3ab_sb[:], in_=w3ab_d[:])
        nc.sync.dma_start(out=w3c_sb[:], in_=w3c_d[:])
        nc.sync.dma_start(out=h_sb[2][44:45, :], in_=m_t[D : D + 1, :])

        conv_group(1, *pending[1])
        pending = pending[2:]

        early_z = os.environ.get("KEARLYZ", "1") == "1"
        if early_z:
            # early MLP z-chunks: mention-only rows ride under the conv;
            # cnn rows (c=0..2) accumulate after the conv.
            for jc, (j0, j1) in enumerate(JCH):
                nc.tensor.matmul(
                    out=z_ps[jc][:], lhsT=wa_sb[:, 3, j0:j1], rhs=c_sb[3][:],
                    start=True, stop=False,
                )
                nc.tensor.matmul(
                    out=z_ps[jc][:], lhsT=wb_sb[:, j0:j1], rhs=c_sb[4][:],
                    start=False, stop=False,
                )

        for g in range(2, NG):
            eg, t1, t2 = gather_group(g)
            conv_group(g, eg, t1, t2)

        # -- MLP tail: z_T = W2^T @ concat_T, h_T = tanh(z_T), out = h_T^T W3
        for jc, (j0, j1) in enumerate(JCH):
            if not early_z:
                nc.tensor.matmul(
                    out=z_ps[jc][:], lhsT=wa_sb[:, 3, j0:j1], rhs=c_sb[3][:],
                    start=True, stop=False,
                )
                nc.tensor.matmul(
                    out=z_ps[jc][:], lhsT=wb_sb[:, j0:j1], rhs=c_sb[4][:],
                    start=False, stop=False,
                )
            for c in range(3):
                nc.tensor.matmul(
                    out=z_ps[jc][:],
                    lhsT=wa_sb[:, c, j0:j1],
                    rhs=c_sb[c][:],
                    start=False,
                    stop=(c == 2),
                )
            rows = h_sb[jc][0 : j1 - j0] if jc == 2 else h_sb[jc][:]
            nc.scalar.activation(
                out=rows[:], in_=z_ps[jc][:], func=mybir.ActivationFunctionType.Tanh
            )

        ps_o = pspool.tile([NS, D], F32, tag="ps", name="po")
        nc.tensor.matmul(
            out=ps_o[:], lhsT=h_sb[0][:], rhs=w3ab_sb[:, 0, :], start=True, stop=False
        )
        nc.tensor.matmul(
            out=ps_o[:], lhsT=h_sb[1][:], rhs=w3ab_sb[:, 1, :], start=False, stop=False
        )
        nc.tensor.matmul(
            out=ps_o[:], lhsT=h_sb[2][:], rhs=w3c_sb[:], start=False, stop=True
        )
        out_sb = const.tile([NS, D], F32)
        nc.scalar.copy(out=out_sb[:], in_=ps_o[:])
        nc.sync.dma_start(out=out_d[:], in_=out_sb[:])

    nc.finalize()
    return nc


def get_program() -> bass.Bass:
    if "v4" not in _PROGRAM_CACHE:
        _PROGRAM_CACHE["v4"] = _build_program()
    return _PROGRAM_CACHE["v4"]


def _prepare_in_maps(inputs: dict) -> list[dict]:
    token_ids = np.asarray(inputs["token_ids"]).astype(np.int64)      # [1024, 128]
    mention = np.asarray(inputs["mention_rep"], dtype=np.float32).reshape(NSENT, D)
    emb = np.asarray(inputs["emb"], dtype=np.float32)
    W1 = np.asarray(inputs["W1"], dtype=np.float64)
    b1 = np.asarray(inputs["b1"], dtype=np.float64)
    conv_w = np.asarray(inputs["conv_w"], dtype=np.float64)           # [o, i, k]
    conv_b = np.asarray(inputs["conv_b"], dtype=np.float64)
    W2 = np.asarray(inputs["W2"], dtype=np.float64)                   # [2D, D]
    b2 = np.asarray(inputs["b2"], dtype=np.float64)
    W3 = np.asarray(inputs["W3"], dtype=np.float64)                   # [j, q]
    b3 = np.asarray(inputs["b3"], dtype=np.float64)

    Wk = conv_w.transpose(1, 0, 2)                                    # [i, o, k]
    weff = np.stack([W1 @ Wk[:, :, k] for k in range(K)])             # [k, i, o]
    beff = b1 @ Wk.sum(axis=2) + conv_b                               # [o]
    b2eff = b2 + beff @ W2[:D]                                        # [j]
    w2cat = np.concatenate([W2, b2eff[None, :]], axis=0)              # [601, 300]
    w3cat = np.concatenate([W3, b3[None, :]], axis=0)                 # [301, 300]

    bf = ml_dtypes.bfloat16
    weff_bf = weff.astype(bf)
    wm_h = np.zeros((128, 10, D), bf)
    for j in range(2):
        for k in range(K):
            wm_h[:, j * 5 + k, :] = weff_bf[k, j * 128 : (j + 1) * 128, :]
    t1w_h = np.zeros((128, D), bf)
    t1w_h[0:44] = weff_bf[0, 256:300]
    t1w_h[44:88] = weff_bf[1, 256:300]
    t1w_h[88:128] = weff_bf[2, 256:296]
    t2w_h = np.zeros((92, D), bf)
    t2w_h[0:4] = weff_bf[2, 296:300]
    t2w_h[4:48] = weff_bf[3, 256:300]
    t2w_h[48:92] = weff_bf[4, 256:300]

    wa_h = np.ascontiguousarray(
        w2cat[:512].reshape(4, 128, D).transpose(1, 0, 2)
    ).astype(np.float32)
    wb_h = w2cat[512:601].astype(np.float32)
    w3ab_h = np.ascontiguousarray(
        w3cat[:256].reshape(2, 128, D).transpose(1, 0, 2)
    ).astype(np.float32)
    w3c_h = w3cat[256:301].astype(np.float32)
    idn_h = np.eye(L, dtype=np.float32)

    in_maps = []
    for c in range(NCORES):
        sl = slice(c * NS, (c + 1) * NS)
        tids = token_ids[sl]                                          # [128, 128]
        uniq, inv = np.unique(tids.ravel(), return_inverse=True)
        assert uniq.size <= UPAD
        tab_h = np.zeros((UPAD, E), bf)
        tab_h[: uniq.size, :D] = emb[uniq].astype(bf)
        # idx wrap: flat i (= local_sent*L + pos) -> col i//16, row i%16,
        # replicated across the 8 partition groups of 16
        cid = inv.reshape(NS, L).astype(np.int16)
        idx_h = np.zeros((128, NG * GCOLS), np.int16)
        for g in range(NG):
            cg = cid[g * GS : (g + 1) * GS].ravel()
            wrap = cg.reshape(GCOLS, 16).T
            idx_h[:, g * GCOLS : (g + 1) * GCOLS] = np.tile(wrap, (8, 1))

        mt_h = np.ones((D + 1, NS), np.float32)
        mt_h[:D] = mention[sl].T
        in_maps.append(
            {
                "tabc": tab_h,
                "idx0": np.ascontiguousarray(idx_h[:, :GCOLS]),
                "idxr": np.ascontiguousarray(idx_h[:, GCOLS:]),
                "wm": wm_h,
                "t1w": t1w_h,
                "t2w": t2w_h,
                "wa": wa_h,
                "wb": wb_h,
                "w3ab": w3ab_h,
                "w3c": w3c_h,
                "idn": idn_h,
                "mt": mt_h,
            }
        )
    return in_maps


def run(inputs: dict, trace: bool = False, **kwargs):
    """Run the kernel; returns (output [1024, 300] f32, BassKernelResults)."""
    nc = get_program()
    in_maps = _prepare_in_maps(inputs)
    res = bass_utils.run_bass_kernel_spmd(
        nc, in_maps, core_ids=list(range(NCORES)), trace=trace, **kwargs
    )
    out = np.concatenate(
        [np.asarray(r["out"]) for r in res.results], axis=0
    ).astype(np.float32)
    return out, res


def kernel(**inputs) -> np.ndarray:
    out, _ = run(inputs)
    return out
